# revision 16
# baseline (speedup 1.0000x reference)
"""MHA kernel for Trainium2, 8-core tensor-parallel (2 heads per core).

Problem (hardcoded): x [2, 2048, 1024] fp32, Wq/Wk/Wv/Wo [1024, 1024],
bq/bk/bv/bo [1024], H=16 heads, DH=64.  out = MHA(x).

Sharding: heads are split 8 ways (2 heads = 128 proj columns per core).
Each core computes its heads' attention output and a partial output
projection (row-parallel Wo); the host sums the 8 partials and adds the
closed-form bias terms (bv @ Wo + bo).

Per-core device pipeline (all big matmuls bf16 in / fp32 accumulate):
  1. Q^T, K^T [128, 4096] = W.T @ x.T            (contract D, psum N=512)
  2. V token-major [tok, 64] per (b, h, ktile), augmented with a ones
     column -> lhsT [128, 65] slots
  3. S^T tiles [128 k, 512 q] = K Q^T; the two heads' K=64 matmuls sit
     on row-groups 0-1 / 2-3 so the PE packs them concurrently
  4. P^T = exp(S^T / 8) on ScalarE (scores in [-3.6, 3.6], no max pass)
  5. O_raw^T + denom = [V|1].T @ P^T             (psum [65, 512])
  6. r2 = 1/denom (both heads), broadcast via one K=2 fp32 matmul
     (eye2), O_norm^T = O_raw^T * r
  7. out^partial [tok 128, 512] = O_norm^T.T @ Wo  (token-major, fp32)
"""

import numpy as np
import ml_dtypes

D = 1024
T = 4096          # B*S tokens
S = 2048
B = 2
NH = 2            # heads per core
DH = 64
NCORES = 8
SCALE = 0.125     # 1/sqrt(DH)

_CACHE = {}


def _build_nc(reps=1):
    import concourse.bacc as bacc
    import concourse.mybir as mybir
    import concourse.tile as tile

    dt = mybir.dt
    f32, bf16 = dt.float32, dt.bfloat16

    nc = bacc.Bacc("TRN2", target_bir_lowering=False, debug=False,
                   num_devices=NCORES)

    xT = nc.dram_tensor("xT", [D, T], bf16, kind="ExternalInput")
    wq_d = nc.dram_tensor("wq", [D, 128], bf16, kind="ExternalInput")
    wk_d = nc.dram_tensor("wk", [D, 128], bf16, kind="ExternalInput")
    wv_d = nc.dram_tensor("wv", [D, 128], bf16, kind="ExternalInput")
    wo_d = nc.dram_tensor("wo", [128, D], bf16, kind="ExternalInput")
    bq_d = nc.dram_tensor("bq", [128, 1], f32, kind="ExternalInput")
    bk_d = nc.dram_tensor("bk", [128, 1], f32, kind="ExternalInput")
    outp = nc.dram_tensor("outp", [T, D], f32, kind="ExternalOutput")

    NKT = S // 128        # 16 key tiles per batch
    NQC = S // 512        # 4 query chunks per batch
    NCK = T // 512        # 8 x^T column chunks
    VSLOT = DH + 1        # 65: V columns + ones column

    with tile.TileContext(nc) as tc:
      for _rep in range(reps):
        with (
            tc.tile_pool(name="persist", bufs=1) as pp,
            tc.tile_pool(name="pt", bufs=4) as ptp,
            tc.tile_pool(name="onorm", bufs=2) as onp,
            tc.tile_pool(name="oraw", bufs=2) as orp,
            tc.tile_pool(name="recip", bufs=2) as rcp,
            tc.tile_pool(name="outsb", bufs=3) as osp,
            tc.tile_pool(name="stage", bufs=4) as stgp,
            tc.tile_pool(name="st_ps", bufs=2, space="PSUM") as stp,
            tc.tile_pool(name="av_ps", bufs=2, space="PSUM") as avp,
            tc.tile_pool(name="mm_ps", bufs=2, space="PSUM") as mmp,
        ):
            # ---- constants / weights ----
            wq = pp.tile([128, D], bf16, tag="wq")
            wk = pp.tile([128, D], bf16, tag="wk")
            wv = pp.tile([128, D], bf16, tag="wv")
            wo = pp.tile([128, D], bf16, tag="wo")
            for w_sb, w_dr in ((wq, wq_d), (wk, wk_d), (wv, wv_d)):
                nc.sync.dma_start(
                    out=w_sb.rearrange("p (t c) -> p t c", c=128),
                    in_=w_dr.ap().rearrange("(t p) c -> p t c", p=128),
                )
            nc.sync.dma_start(out=wo[:, :], in_=wo_d.ap()[:, :])
            bq = pp.tile([128, 1], f32, tag="bq")
            bk = pp.tile([128, 1], f32, tag="bk")
            nc.sync.dma_start(out=bq[:, :], in_=bq_d.ap()[:, :])
            nc.sync.dma_start(out=bk[:, :], in_=bk_d.ap()[:, :])

            # ---- x^T d-tiles, loaded in 512-col chunks so the QKV
            # matmuls can start as soon as chunk 0 of all 8 d-tiles lands
            xt = [pp.tile([128, T], bf16, tag=f"xt{d}", name=f"xt{d}")
                  for d in range(8)]
            for nck in range(NCK):
                cs = slice(nck * 512, (nck + 1) * 512)
                for d in range(8):
                    nc.sync.dma_start(
                        out=xt[d][:, cs],
                        in_=xT.ap()[d * 128:(d + 1) * 128, cs])

            # ---- Q^T / K^T projections ----
            qt = pp.tile([128, T], bf16, tag="qt")
            kt = pp.tile([128, T], bf16, tag="kt")
            for nck in range(NCK):
                cs = slice(nck * 512, (nck + 1) * 512)
                for proj_sb, w_sb, b_sb in ((qt, wq, bq), (kt, wk, bk)):
                    w3 = w_sb.rearrange("p (t c) -> p t c", c=128)
                    ps = mmp.tile([128, 512], f32, tag="mm")
                    for d in range(8):
                        nc.tensor.matmul(
                            ps[:, :], w3[:, d, :], xt[d][:, cs],
                            start=(d == 0), stop=(d == 7),
                        )
                    nc.vector.tensor_scalar_add(proj_sb[:, cs], ps[:, :],
                                                b_sb[:, :])

            # ---- V token-major (augmented with ones col) ----
            # per batch: [128 tok, (h, kt) slots of 65]
            vtm = []
            for b in range(B):
                v_sb = pp.tile([128, NH * NKT * VSLOT], bf16, tag=f"v{b}")
                v4 = v_sb.rearrange("p (h k c) -> p h k c", h=NH, k=NKT)
                nc.vector.memset(v4[:, :, :, DH:DH + 1], 1.0)
                vtm.append(v_sb)
            wv3 = wv.rearrange("p (t c) -> p t c", c=128)
            for b in range(B):
                v4 = vtm[b].rearrange("p (h k c) -> p h k c", h=NH, k=NKT)
                for k in range(NKT):
                    tok0 = b * S + k * 128
                    ps = mmp.tile([128, 128], f32, tag="mm")
                    for d in range(8):
                        nc.tensor.matmul(
                            ps[:, :], xt[d][:, tok0:tok0 + 128], wv3[:, d, :],
                            start=(d == 0), stop=(d == 7),
                        )
                    nc.vector.tensor_copy(
                        v4[:, :, k, 0:DH],
                        ps.rearrange("p (h c) -> p h c", h=NH)[:, :, :],
                    )

            # ---- attention per (b, qc), heads interleaved ----
            # The outproj matmuls of combo i are emitted AFTER combo i+1's
            # scores/AV matmuls: PE is in-order, and this gives the
            # reciprocal/broadcast chain a full combo to finish without
            # stalling the PE (stalls > 3.4us re-throttle the PE clock).
            def emit_outproj(q0, onorm):
                for s4 in range(4):
                    for jc in range(2):
                        op = mmp.tile([128, 512], f32, tag="mm",
                                      name=f"op{q0}_{s4}_{jc}")
                        nc.tensor.matmul(
                            op[:, :], onorm[:, s4 * 128:(s4 + 1) * 128],
                            wo[:, jc * 512:(jc + 1) * 512],
                            start=True, stop=True,
                        )
                        osb = osp.tile([128, 512], f32, tag="outsb",
                                       name=f"osb{q0}_{s4}_{jc}")
                        if jc == 0:
                            nc.vector.tensor_copy(osb[:, :], op[:, :])
                        else:
                            nc.scalar.copy(osb[:, :], op[:, :])
                        r0 = q0 + s4 * 128
                        nc.sync.dma_start(
                            out=outp.ap()[r0:r0 + 128,
                                          jc * 512:(jc + 1) * 512],
                            in_=osb[:, :],
                        )

            pending = None
            for b in range(B):
                v4 = vtm[b].rearrange("p (h k c) -> p h k c", h=NH, k=NKT)
                for qc in range(NQC):
                    q0 = b * S + qc * 512
                    onorm = onp.tile([128, 512], bf16, tag="onorm",
                                     name=f"onorm{b}_{qc}")
                    pts = [ptp.tile([128, NKT * 512], bf16, tag="pt",
                                    name=f"pt{b}_{qc}_{h}")
                           for h in range(NH)]
                    # scores + exp, alternating heads so the two heads'
                    # K=64 matmuls co-reside in the PE queue (row-packing)
                    for g in range(NKT // 2):
                        for h in range(NH):
                            hp = h * DH
                            st = stp.tile([128, 1024], f32, tag="st")
                            for i in range(2):
                                k0 = b * S + (2 * g + i) * 128
                                nc.tensor.matmul(
                                    st[:, i * 512:(i + 1) * 512],
                                    kt[hp:hp + DH, k0:k0 + 128],
                                    qt[hp:hp + DH, q0:q0 + 512],
                                    start=True, stop=True,
                                )
                            nc.scalar.activation(
                                pts[h][:, g * 1024:(g + 1) * 1024], st[:, :],
                                mybir.ActivationFunctionType.Exp,
                                scale=SCALE,
                            )
                    for h in range(NH):
                        hp = h * DH
                        av = avp.tile([128, 512], f32, tag="av")
                        for k in range(NKT):
                            nc.tensor.matmul(
                                av[0:VSLOT, :], v4[:, h, k, :],
                                pts[h][:, k * 512:(k + 1) * 512],
                                start=(k == 0), stop=(k == NKT - 1),
                            )
                        oraw = orp.tile([VSLOT, 512], f32, tag="oraw")
                        nc.vector.tensor_copy(oraw[:, :], av[0:VSLOT, :])
                        recip = rcp.tile([1, 512], f32, tag="recip")
                        nc.vector.reciprocal(recip[:, :],
                                             oraw[DH:VSLOT, :])
                        rb = orp.tile([DH, 512], f32, tag="rb")
                        nc.gpsimd.partition_broadcast(rb[:, :], recip[:, :])
                        nc.vector.tensor_tensor(
                            onorm[hp:hp + DH, :], oraw[0:DH, :],
                            rb[:, :], op=mybir.AluOpType.mult,
                        )
                    if pending is not None:
                        emit_outproj(*pending)
                    pending = (q0, onorm)
            emit_outproj(*pending)

    nc.compile()
    return nc


def _prep_inputs(x, Wq, bq, Wk, bk, Wv, bv, Wo, bo):
    bf16 = ml_dtypes.bfloat16
    xT = np.ascontiguousarray(
        np.asarray(x, dtype=np.float32).reshape(T, D).T).astype(bf16)
    in_maps = []
    for c in range(NCORES):
        cs = slice(c * 128, (c + 1) * 128)
        in_maps.append({
            "xT": xT,
            "wq": np.ascontiguousarray(Wq[:, cs]).astype(bf16),
            "wk": np.ascontiguousarray(Wk[:, cs]).astype(bf16),
            "wv": np.ascontiguousarray(Wv[:, cs]).astype(bf16),
            "wo": np.ascontiguousarray(Wo[cs, :]).astype(bf16),
            "bq": np.ascontiguousarray(bq[cs]).reshape(128, 1).astype(np.float32),
            "bk": np.ascontiguousarray(bk[cs]).reshape(128, 1).astype(np.float32),
        })
    return in_maps


def kernel(x, Wq, bq, Wk, bk, Wv, bv, Wo, bo, _trace=False, _results=None):
    from concourse.bass_utils import run_bass_kernel_spmd

    x = np.asarray(x); Wq = np.asarray(Wq); Wk = np.asarray(Wk)
    Wv = np.asarray(Wv); Wo = np.asarray(Wo)
    bq = np.asarray(bq); bk = np.asarray(bk); bv = np.asarray(bv)
    bo = np.asarray(bo)

    if "nc" not in _CACHE:
        _CACHE["nc"] = _build_nc()
    nc = _CACHE["nc"]

    in_maps = _prep_inputs(x, Wq, bq, Wk, bk, Wv, bv, Wo, bo)
    res = run_bass_kernel_spmd(
        nc, in_maps, core_ids=list(range(NCORES)), trace=_trace)
    if _results is not None:
        _results.append(res)

    acc = np.zeros((T, D), dtype=np.float32)
    for c in range(NCORES):
        acc += np.asarray(res.results[c]["outp"], dtype=np.float32)
    acc += bv.astype(np.float32) @ Wo.astype(np.float32) + bo.astype(np.float32)
    return acc.reshape(B, S, D)


# revision 19
# speedup vs baseline: 1.1944x; 1.1944x over previous
"""MHA kernel for Trainium2, 8-core tensor-parallel (2 heads per core).

Problem (hardcoded): x [2, 2048, 1024] fp32, Wq/Wk/Wv/Wo [1024, 1024],
bq/bk/bv/bo [1024], H=16 heads, DH=64.  out = MHA(x).

Sharding: heads are split 8 ways (2 heads = 128 proj columns per core).
Each core computes its heads' attention output and a partial output
projection (row-parallel Wo); the host sums the 8 partials and adds the
closed-form bias terms (bv @ Wo + bo).

Per-core device pipeline (all big matmuls bf16 in / fp32 accumulate):
  1. Q^T, K^T [128, 4096] = W.T @ x.T            (contract D, psum N=512)
  2. V token-major [tok, 64] per (b, h, ktile), augmented with a ones
     column -> lhsT [128, 65] slots
  3. S^T tiles [128 k, 512 q] = K Q^T; the two heads' K=64 matmuls sit
     on row-groups 0-1 / 2-3 so the PE packs them concurrently
  4. P^T = exp(S^T / 8) on ScalarE (scores in [-3.6, 3.6], no max pass)
  5. O_raw^T + denom = [V|1].T @ P^T             (psum [65, 512])
  6. r2 = 1/denom (both heads), broadcast via one K=2 fp32 matmul
     (eye2), O_norm^T = O_raw^T * r
  7. out^partial [tok 128, 512] = O_norm^T.T @ Wo  (token-major, fp32)
"""

import numpy as np
import ml_dtypes

D = 1024
T = 4096          # B*S tokens
S = 2048
B = 2
NH = 2            # heads per core
DH = 64
NCORES = 8
SCALE = 0.125     # 1/sqrt(DH)

_CACHE = {}


def _build_nc(reps=1):
    import concourse.bacc as bacc
    import concourse.mybir as mybir
    import concourse.tile as tile

    dt = mybir.dt
    f32, bf16 = dt.float32, dt.bfloat16

    nc = bacc.Bacc("TRN2", target_bir_lowering=False, debug=False,
                   num_devices=NCORES)

    xT = nc.dram_tensor("xT", [D, T], bf16, kind="ExternalInput")
    wq_d = nc.dram_tensor("wq", [D, 128], bf16, kind="ExternalInput")
    wk_d = nc.dram_tensor("wk", [D, 128], bf16, kind="ExternalInput")
    wv_d = nc.dram_tensor("wv", [D, 128], bf16, kind="ExternalInput")
    wo_d = nc.dram_tensor("wo", [128, D], bf16, kind="ExternalInput")
    bq_d = nc.dram_tensor("bq", [128, 1], f32, kind="ExternalInput")
    bk_d = nc.dram_tensor("bk", [128, 1], f32, kind="ExternalInput")
    outp = nc.dram_tensor("outp", [T, D], f32, kind="ExternalOutput")

    NKT = S // 128        # 16 key tiles per batch
    NQC = S // 512        # 4 query chunks per batch
    NCK = T // 512        # 8 x^T column chunks
    VSLOT = DH + 1        # 65: V columns + ones column

    with tile.TileContext(nc) as tc:
      for _rep in range(reps):
        with (
            tc.tile_pool(name="persist", bufs=1) as pp,
            tc.tile_pool(name="pt", bufs=2) as ptp,
            tc.tile_pool(name="onorm", bufs=2) as onp,
            tc.tile_pool(name="oraw", bufs=2) as orp,
            tc.tile_pool(name="recip", bufs=2) as rcp,
            tc.tile_pool(name="outsb", bufs=3) as osp,
            tc.tile_pool(name="st_ps", bufs=2, space="PSUM") as stp,
            tc.tile_pool(name="av_ps", bufs=2, space="PSUM") as avp,
            tc.tile_pool(name="mm_ps", bufs=2, space="PSUM") as mmp,
        ):
            # ---- constants / weights ----
            wq = pp.tile([128, D], bf16, tag="wq")
            wk = pp.tile([128, D], bf16, tag="wk")
            wv = pp.tile([128, D], bf16, tag="wv")
            wo = pp.tile([128, D], bf16, tag="wo")
            for w_sb, w_dr in ((wq, wq_d), (wk, wk_d), (wv, wv_d)):
                nc.sync.dma_start(
                    out=w_sb.rearrange("p (t c) -> p t c", c=128),
                    in_=w_dr.ap().rearrange("(t p) c -> p t c", p=128),
                )
            nc.sync.dma_start(out=wo[:, :], in_=wo_d.ap()[:, :])
            bq = pp.tile([128, 1], f32, tag="bq")
            bk = pp.tile([128, 1], f32, tag="bk")
            nc.sync.dma_start(out=bq[:, :], in_=bq_d.ap()[:, :])
            nc.sync.dma_start(out=bk[:, :], in_=bk_d.ap()[:, :])

            # ---- x^T d-tiles, loaded in 512-col chunks so the QKV
            # matmuls can start as soon as chunk 0 of all 8 d-tiles lands
            xt = [pp.tile([128, T], bf16, tag=f"xt{d}", name=f"xt{d}")
                  for d in range(8)]
            for nck in range(NCK):
                cs = slice(nck * 512, (nck + 1) * 512)
                for d in range(8):
                    nc.sync.dma_start(
                        out=xt[d][:, cs],
                        in_=xT.ap()[d * 128:(d + 1) * 128, cs])

            # ---- Q^T / K^T projections ----
            qt = pp.tile([128, T], bf16, tag="qt")
            kt = pp.tile([128, T], bf16, tag="kt")
            for nck in range(NCK):
                cs = slice(nck * 512, (nck + 1) * 512)
                for proj_sb, w_sb, b_sb in ((qt, wq, bq), (kt, wk, bk)):
                    w3 = w_sb.rearrange("p (t c) -> p t c", c=128)
                    ps = mmp.tile([128, 512], f32, tag="mm")
                    for d in range(8):
                        nc.tensor.matmul(
                            ps[:, :], w3[:, d, :], xt[d][:, cs],
                            start=(d == 0), stop=(d == 7),
                        )
                    nc.vector.tensor_scalar_add(proj_sb[:, cs], ps[:, :],
                                                b_sb[:, :])

            # ---- V token-major (augmented with ones col) ----
            # per batch: [128 tok, (h, kt) slots of 65]
            vtm = []
            for b in range(B):
                v_sb = pp.tile([128, NH * NKT * VSLOT], bf16, tag=f"v{b}")
                v4 = v_sb.rearrange("p (h k c) -> p h k c", h=NH, k=NKT)
                nc.vector.memset(v4[:, :, :, DH:DH + 1], 1.0)
                vtm.append(v_sb)
            wv3 = wv.rearrange("p (t c) -> p t c", c=128)
            for b in range(B):
                v4 = vtm[b].rearrange("p (h k c) -> p h k c", h=NH, k=NKT)
                for k in range(NKT):
                    tok0 = b * S + k * 128
                    ps = mmp.tile([128, 128], f32, tag="mm")
                    for d in range(8):
                        nc.tensor.matmul(
                            ps[:, :], xt[d][:, tok0:tok0 + 128], wv3[:, d, :],
                            start=(d == 0), stop=(d == 7),
                        )
                    nc.vector.tensor_copy(
                        v4[:, :, k, 0:DH],
                        ps.rearrange("p (h c) -> p h c", h=NH)[:, :, :],
                    )

            # ---- attention per (b, qc), heads interleaved ----
            # The outproj matmuls of combo i are emitted AFTER combo i+1's
            # scores/AV matmuls: PE is in-order, and this gives the
            # reciprocal/broadcast chain a full combo to finish without
            # stalling the PE (stalls > 3.4us re-throttle the PE clock).
            def emit_outproj(q0, onorm):
                for s4 in range(4):
                    for jc in range(2):
                        op = mmp.tile([128, 512], f32, tag="mm",
                                      name=f"op{q0}_{s4}_{jc}")
                        nc.tensor.matmul(
                            op[:, :], onorm[:, s4 * 128:(s4 + 1) * 128],
                            wo[:, jc * 512:(jc + 1) * 512],
                            start=True, stop=True,
                        )
                        osb = osp.tile([128, 512], f32, tag="outsb",
                                       name=f"osb{q0}_{s4}_{jc}")
                        if jc == 0:
                            nc.vector.tensor_copy(osb[:, :], op[:, :])
                        else:
                            nc.scalar.copy(osb[:, :], op[:, :])
                        r0 = q0 + s4 * 128
                        nc.sync.dma_start(
                            out=outp.ap()[r0:r0 + 128,
                                          jc * 512:(jc + 1) * 512],
                            in_=osb[:, :],
                        )

            pending = None
            for b in range(B):
                v4 = vtm[b].rearrange("p (h k c) -> p h k c", h=NH, k=NKT)
                for qc in range(NQC):
                    q0 = b * S + qc * 512
                    onorm = onp.tile([128, 512], bf16, tag="onorm",
                                     name=f"onorm{b}_{qc}")
                    pt = ptp.tile([128, NH * NKT * 512], bf16, tag="pt",
                                  name=f"pt{b}_{qc}")
                    pt3 = pt.rearrange("p (h k q) -> p h k q", h=NH, k=NKT)
                    # scores: each st tile holds both heads for one ktile;
                    # consecutive matmuls alternate PE row-groups (0-63 vs
                    # 64-127) so the array runs them concurrently
                    for kti in range(NKT):
                        k0 = b * S + kti * 128
                        st = stp.tile([128, 1024], f32, tag="st")
                        for h in range(NH):
                            hp = h * DH
                            nc.tensor.matmul(
                                st[:, h * 512:(h + 1) * 512],
                                kt[hp:hp + DH, k0:k0 + 128],
                                qt[hp:hp + DH, q0:q0 + 512],
                                start=True, stop=True,
                            )
                        nc.scalar.activation(
                            pt3[:, :, kti, :], st[:, :],
                            mybir.ActivationFunctionType.Exp,
                            scale=SCALE,
                        )
                    for h in range(NH):
                        hp = h * DH
                        av = avp.tile([128, 512], f32, tag="av")
                        for k in range(NKT):
                            nc.tensor.matmul(
                                av[0:VSLOT, :], v4[:, h, k, :],
                                pt3[:, h, k, :],
                                start=(k == 0), stop=(k == NKT - 1),
                            )
                        oraw = orp.tile([VSLOT, 512], f32, tag="oraw")
                        nc.vector.tensor_copy(oraw[:, :], av[0:VSLOT, :])
                        recip = rcp.tile([1, 512], f32, tag="recip")
                        nc.vector.reciprocal(recip[:, :],
                                             oraw[DH:VSLOT, :])
                        rb = orp.tile([DH, 512], f32, tag="rb")
                        nc.gpsimd.partition_broadcast(rb[:, :], recip[:, :])
                        nc.vector.tensor_tensor(
                            onorm[hp:hp + DH, :], oraw[0:DH, :],
                            rb[:, :], op=mybir.AluOpType.mult,
                        )
                    if pending is not None:
                        emit_outproj(*pending)
                    pending = (q0, onorm)
            emit_outproj(*pending)

    nc.compile()
    return nc


def _prep_inputs(x, Wq, bq, Wk, bk, Wv, bv, Wo, bo):
    bf16 = ml_dtypes.bfloat16
    xT = np.ascontiguousarray(
        np.asarray(x, dtype=np.float32).reshape(T, D).T).astype(bf16)
    in_maps = []
    for c in range(NCORES):
        cs = slice(c * 128, (c + 1) * 128)
        in_maps.append({
            "xT": xT,
            "wq": np.ascontiguousarray(Wq[:, cs]).astype(bf16),
            "wk": np.ascontiguousarray(Wk[:, cs]).astype(bf16),
            "wv": np.ascontiguousarray(Wv[:, cs]).astype(bf16),
            "wo": np.ascontiguousarray(Wo[cs, :]).astype(bf16),
            "bq": np.ascontiguousarray(bq[cs]).reshape(128, 1).astype(np.float32),
            "bk": np.ascontiguousarray(bk[cs]).reshape(128, 1).astype(np.float32),
        })
    return in_maps


def kernel(x, Wq, bq, Wk, bk, Wv, bv, Wo, bo, _trace=False, _results=None):
    from concourse.bass_utils import run_bass_kernel_spmd

    x = np.asarray(x); Wq = np.asarray(Wq); Wk = np.asarray(Wk)
    Wv = np.asarray(Wv); Wo = np.asarray(Wo)
    bq = np.asarray(bq); bk = np.asarray(bk); bv = np.asarray(bv)
    bo = np.asarray(bo)

    if "nc" not in _CACHE:
        _CACHE["nc"] = _build_nc()
    nc = _CACHE["nc"]

    in_maps = _prep_inputs(x, Wq, bq, Wk, bk, Wv, bv, Wo, bo)
    res = run_bass_kernel_spmd(
        nc, in_maps, core_ids=list(range(NCORES)), trace=_trace)
    if _results is not None:
        _results.append(res)

    acc = np.zeros((T, D), dtype=np.float32)
    for c in range(NCORES):
        acc += np.asarray(res.results[c]["outp"], dtype=np.float32)
    acc += bv.astype(np.float32) @ Wo.astype(np.float32) + bo.astype(np.float32)
    return acc.reshape(B, S, D)


# revision 20
# speedup vs baseline: 1.2522x; 1.0484x over previous
"""MHA kernel for Trainium2, 8-core tensor-parallel (2 heads per core).

Problem (hardcoded): x [2, 2048, 1024] fp32, Wq/Wk/Wv/Wo [1024, 1024],
bq/bk/bv/bo [1024], H=16 heads, DH=64.  out = MHA(x).

Sharding: heads are split 8 ways (2 heads = 128 proj columns per core).
Each core computes its heads' attention output and a partial output
projection (row-parallel Wo); the host sums the 8 partials and adds the
closed-form bias terms (bv @ Wo + bo).

Per-core device pipeline (all big matmuls bf16 in / fp32 accumulate):
  1. Q^T, K^T [128, 4096] = W.T @ x.T            (contract D, psum N=512)
  2. V token-major [tok, 64] per (b, h, ktile), augmented with a ones
     column -> lhsT [128, 65] slots
  3. S^T tiles [128 k, 512 q] = K Q^T; the two heads' K=64 matmuls sit
     on row-groups 0-1 / 2-3 so the PE packs them concurrently
  4. P^T = exp(S^T / 8) on ScalarE (scores in [-3.6, 3.6], no max pass)
  5. O_raw^T + denom = [V|1].T @ P^T             (psum [65, 512])
  6. r2 = 1/denom (both heads), broadcast via one K=2 fp32 matmul
     (eye2), O_norm^T = O_raw^T * r
  7. out^partial [tok 128, 512] = O_norm^T.T @ Wo  (token-major, fp32)
"""

import numpy as np
import ml_dtypes

D = 1024
T = 4096          # B*S tokens
S = 2048
B = 2
NH = 2            # heads per core
DH = 64
NCORES = 8
SCALE = 0.125     # 1/sqrt(DH)

_CACHE = {}


def _build_nc(reps=1):
    import concourse.bacc as bacc
    import concourse.mybir as mybir
    import concourse.tile as tile

    dt = mybir.dt
    f32, bf16 = dt.float32, dt.bfloat16

    nc = bacc.Bacc("TRN2", target_bir_lowering=False, debug=False,
                   num_devices=NCORES)

    xT = nc.dram_tensor("xT", [D, T], bf16, kind="ExternalInput")
    wq_d = nc.dram_tensor("wq", [D, 128], bf16, kind="ExternalInput")
    wk_d = nc.dram_tensor("wk", [D, 128], bf16, kind="ExternalInput")
    wv_d = nc.dram_tensor("wv", [D, 128], bf16, kind="ExternalInput")
    wo_d = nc.dram_tensor("wo", [128, D], bf16, kind="ExternalInput")
    bq_d = nc.dram_tensor("bq", [128, 1], f32, kind="ExternalInput")
    bk_d = nc.dram_tensor("bk", [128, 1], f32, kind="ExternalInput")
    outp = nc.dram_tensor("outp", [T, D], f32, kind="ExternalOutput")

    NKT = S // 128        # 16 key tiles per batch
    NQC = S // 512        # 4 query chunks per batch
    NCK = T // 512        # 8 x^T column chunks
    VSLOT = DH + 1        # 65: V columns + ones column

    with tile.TileContext(nc) as tc:
      for _rep in range(reps):
        with (
            tc.tile_pool(name="persist", bufs=1) as pp,
            tc.tile_pool(name="pt", bufs=2) as ptp,
            tc.tile_pool(name="onorm", bufs=2) as onp,
            tc.tile_pool(name="oraw", bufs=2) as orp,
            tc.tile_pool(name="recip", bufs=2) as rcp,
            tc.tile_pool(name="outsb", bufs=3) as osp,
            tc.tile_pool(name="st_ps", bufs=2, space="PSUM") as stp,
            tc.tile_pool(name="av_ps", bufs=2, space="PSUM") as avp,
            tc.tile_pool(name="mm_ps", bufs=2, space="PSUM") as mmp,
        ):
            # ---- constants / weights ----
            wq = pp.tile([128, D], bf16, tag="wq")
            wk = pp.tile([128, D], bf16, tag="wk")
            wv = pp.tile([128, D], bf16, tag="wv")
            wo = pp.tile([128, D], bf16, tag="wo")
            for w_sb, w_dr in ((wq, wq_d), (wk, wk_d), (wv, wv_d)):
                nc.sync.dma_start(
                    out=w_sb.rearrange("p (t c) -> p t c", c=128),
                    in_=w_dr.ap().rearrange("(t p) c -> p t c", p=128),
                )
            nc.sync.dma_start(out=wo[:, :], in_=wo_d.ap()[:, :])
            bq = pp.tile([128, 1], f32, tag="bq")
            bk = pp.tile([128, 1], f32, tag="bk")
            nc.sync.dma_start(out=bq[:, :], in_=bq_d.ap()[:, :])
            nc.sync.dma_start(out=bk[:, :], in_=bk_d.ap()[:, :])

            # ---- x^T d-tiles, loaded in 512-col chunks so the QKV
            # matmuls can start as soon as chunk 0 of all 8 d-tiles lands
            xt = [pp.tile([128, T], bf16, tag=f"xt{d}", name=f"xt{d}")
                  for d in range(8)]
            for nck in range(NCK):
                cs = slice(nck * 512, (nck + 1) * 512)
                for d in range(8):
                    nc.sync.dma_start(
                        out=xt[d][:, cs],
                        in_=xT.ap()[d * 128:(d + 1) * 128, cs])

            # ---- Q^T / K^T projections ----
            qt = pp.tile([128, T], bf16, tag="qt")
            kt = pp.tile([128, T], bf16, tag="kt")
            for nck in range(NCK):
                cs = slice(nck * 512, (nck + 1) * 512)
                for proj_sb, w_sb, b_sb in ((qt, wq, bq), (kt, wk, bk)):
                    w3 = w_sb.rearrange("p (t c) -> p t c", c=128)
                    ps = mmp.tile([128, 512], f32, tag="mm")
                    for d in range(8):
                        nc.tensor.matmul(
                            ps[:, :], w3[:, d, :], xt[d][:, cs],
                            start=(d == 0), stop=(d == 7),
                        )
                    nc.vector.tensor_scalar_add(proj_sb[:, cs], ps[:, :],
                                                b_sb[:, :])

            # ---- V token-major (augmented with ones col) ----
            # per batch: [128 tok, (h, kt) slots of 65]
            vtm = []
            for b in range(B):
                v_sb = pp.tile([128, NH * NKT * VSLOT], bf16, tag=f"v{b}")
                v4 = v_sb.rearrange("p (h k c) -> p h k c", h=NH, k=NKT)
                nc.vector.memset(v4[:, :, :, DH:DH + 1], 1.0)
                vtm.append(v_sb)
            wv3 = wv.rearrange("p (t c) -> p t c", c=128)
            for b in range(B):
                v4 = vtm[b].rearrange("p (h k c) -> p h k c", h=NH, k=NKT)
                for k in range(NKT):
                    tok0 = b * S + k * 128
                    ps = mmp.tile([128, 128], f32, tag="mm")
                    for d in range(8):
                        nc.tensor.matmul(
                            ps[:, :], xt[d][:, tok0:tok0 + 128], wv3[:, d, :],
                            start=(d == 0), stop=(d == 7),
                        )
                    nc.vector.tensor_copy(
                        v4[:, :, k, 0:DH],
                        ps.rearrange("p (h c) -> p h c", h=NH)[:, :, :],
                    )

            # ---- attention per (b, qc), heads interleaved ----
            # The outproj matmuls of combo i are emitted AFTER combo i+1's
            # scores/AV matmuls: PE is in-order, and this gives the
            # reciprocal/broadcast chain a full combo to finish without
            # stalling the PE (stalls > 3.4us re-throttle the PE clock).
            def emit_outproj(q0, onorm):
                for s4 in range(4):
                    for jc in range(2):
                        op = mmp.tile([128, 512], f32, tag="mm",
                                      name=f"op{q0}_{s4}_{jc}")
                        nc.tensor.matmul(
                            op[:, :], onorm[:, s4 * 128:(s4 + 1) * 128],
                            wo[:, jc * 512:(jc + 1) * 512],
                            start=True, stop=True,
                        )
                        osb = osp.tile([128, 512], f32, tag="outsb",
                                       name=f"osb{q0}_{s4}_{jc}")
                        nc.vector.tensor_copy(osb[:, :], op[:, :])
                        r0 = q0 + s4 * 128
                        nc.sync.dma_start(
                            out=outp.ap()[r0:r0 + 128,
                                          jc * 512:(jc + 1) * 512],
                            in_=osb[:, :],
                        )

            pending = None
            for b in range(B):
                v4 = vtm[b].rearrange("p (h k c) -> p h k c", h=NH, k=NKT)
                for qc in range(NQC):
                    q0 = b * S + qc * 512
                    onorm = onp.tile([128, 512], bf16, tag="onorm",
                                     name=f"onorm{b}_{qc}")
                    pt = ptp.tile([128, NH * NKT * 512], bf16, tag="pt",
                                  name=f"pt{b}_{qc}")
                    pt3 = pt.rearrange("p (h k q) -> p h k q", h=NH, k=NKT)
                    # scores: each st tile holds both heads for one ktile;
                    # consecutive matmuls alternate PE row-groups (0-63 vs
                    # 64-127) so the array runs them concurrently
                    for kti in range(NKT):
                        k0 = b * S + kti * 128
                        st = stp.tile([128, 1024], f32, tag="st")
                        for h in range(NH):
                            hp = h * DH
                            nc.tensor.matmul(
                                st[:, h * 512:(h + 1) * 512],
                                kt[hp:hp + DH, k0:k0 + 128],
                                qt[hp:hp + DH, q0:q0 + 512],
                                start=True, stop=True,
                            )
                        nc.scalar.activation(
                            pt3[:, :, kti, :], st[:, :],
                            mybir.ActivationFunctionType.Exp,
                            scale=SCALE,
                        )
                    for h in range(NH):
                        hp = h * DH
                        av = avp.tile([128, 512], f32, tag="av")
                        for k in range(NKT):
                            nc.tensor.matmul(
                                av[0:VSLOT, :], v4[:, h, k, :],
                                pt3[:, h, k, :],
                                start=(k == 0), stop=(k == NKT - 1),
                            )
                        oraw = orp.tile([VSLOT, 512], f32, tag="oraw")
                        nc.vector.tensor_copy(oraw[:, :], av[0:VSLOT, :])
                        negl = rcp.tile([1, 512], f32, tag="negl")
                        nc.scalar.activation(
                            negl[:, :], oraw[DH:VSLOT, :],
                            mybir.ActivationFunctionType.Ln)
                        recip = rcp.tile([1, 512], f32, tag="recip")
                        nc.scalar.activation(
                            recip[:, :], negl[:, :],
                            mybir.ActivationFunctionType.Exp, scale=-1.0)
                        rb = orp.tile([DH, 512], f32, tag="rb")
                        nc.gpsimd.partition_broadcast(rb[:, :], recip[:, :])
                        nc.vector.tensor_tensor(
                            onorm[hp:hp + DH, :], oraw[0:DH, :],
                            rb[:, :], op=mybir.AluOpType.mult,
                        )
                    if pending is not None:
                        emit_outproj(*pending)
                    pending = (q0, onorm)
            emit_outproj(*pending)

    nc.compile()
    return nc


def _prep_inputs(x, Wq, bq, Wk, bk, Wv, bv, Wo, bo):
    bf16 = ml_dtypes.bfloat16
    xT = np.ascontiguousarray(
        np.asarray(x, dtype=np.float32).reshape(T, D).T).astype(bf16)
    in_maps = []
    for c in range(NCORES):
        cs = slice(c * 128, (c + 1) * 128)
        in_maps.append({
            "xT": xT,
            "wq": np.ascontiguousarray(Wq[:, cs]).astype(bf16),
            "wk": np.ascontiguousarray(Wk[:, cs]).astype(bf16),
            "wv": np.ascontiguousarray(Wv[:, cs]).astype(bf16),
            "wo": np.ascontiguousarray(Wo[cs, :]).astype(bf16),
            "bq": np.ascontiguousarray(bq[cs]).reshape(128, 1).astype(np.float32),
            "bk": np.ascontiguousarray(bk[cs]).reshape(128, 1).astype(np.float32),
        })
    return in_maps


def kernel(x, Wq, bq, Wk, bk, Wv, bv, Wo, bo, _trace=False, _results=None):
    from concourse.bass_utils import run_bass_kernel_spmd

    x = np.asarray(x); Wq = np.asarray(Wq); Wk = np.asarray(Wk)
    Wv = np.asarray(Wv); Wo = np.asarray(Wo)
    bq = np.asarray(bq); bk = np.asarray(bk); bv = np.asarray(bv)
    bo = np.asarray(bo)

    if "nc" not in _CACHE:
        _CACHE["nc"] = _build_nc()
    nc = _CACHE["nc"]

    in_maps = _prep_inputs(x, Wq, bq, Wk, bk, Wv, bv, Wo, bo)
    res = run_bass_kernel_spmd(
        nc, in_maps, core_ids=list(range(NCORES)), trace=_trace)
    if _results is not None:
        _results.append(res)

    acc = np.zeros((T, D), dtype=np.float32)
    for c in range(NCORES):
        acc += np.asarray(res.results[c]["outp"], dtype=np.float32)
    acc += bv.astype(np.float32) @ Wo.astype(np.float32) + bo.astype(np.float32)
    return acc.reshape(B, S, D)
